# revision 7
# baseline (speedup 1.0000x reference)
"""MoE gate kernel for TRN2 (8 NeuronCores, Bass/Tile).

Computes, for hidden_states [8, 8192, 512] f32 and gate weight [4, 512] f32:
  logits = x @ W^T, scores = softmax(logits), top-2 (values normalized) and
  the seq-aux load-balancing loss — matching the reference MoEGate module.

Sharding: data-parallel over the batch dim — core c handles batch row c
(8192 tokens). The tiny weight is replicated. The scalar aux_loss partials
([scores_colsum, expert_counts] per core) are combined on the host.

Per-core dataflow, token t = p*64 + i (p = SBUF partition, i = tile 0..63):
  phase 1 (per 128-token tile): DMA x in 1 MiB chunks -> PE transpose chunks
  [128t,128h] -> [128h,128t] (fp32 transpose is bit-exact) -> 4 accumulating
  fp32 matmuls against W^T chunks -> logits [128,4] in PSUM -> batched copy
  into an SBUF plane buffer logits_all [128, 4*64] (plane e at cols 64e+i).
  phase 2 (all tokens at once, [128,64] plane ops): top-2 via min/max
  identities, argmax via is_equal masks, weights via exp + reciprocal,
  softmax column-sums and expert counts reduced per partition, then one
  ones-vector matmul to reduce across partitions.
"""

import numpy as np

import concourse.bass as bass
import concourse.tile as tile
from concourse import mybir
from concourse.bass_utils import run_bass_kernel_spmd

N_CORES = 8
BSZ, SEQ, H = 8, 8192, 512
E = 4  # experts
K = 2  # top-k
ALPHA = 0.1
T = SEQ  # tokens per core
P = 128  # partitions
NT = T // P  # 64 token-tiles per core
DMA_GROUP = 4  # token-tiles per input DMA (1 MiB)
LG_BATCH = 16  # token-tiles per PSUM logits batch
NB = NT // LG_BATCH  # 4 phase-2 batches
Bq = LG_BATCH  # columns per batch in plane space
NB_AUX = 2 * 4  # aux partial cols per batch

F32 = mybir.dt.float32
I32 = mybir.dt.int32
Alu = mybir.AluOpType
Act = mybir.ActivationFunctionType


def _split_excess_waits(nc):
    """walrus in this container allows 1 sync wait per instruction (2 for
    EventSemaphore); Tile's final drain can carry more. Move extras to NOPs."""
    caps = {"InstEventSemaphore": 2}
    for bbname, bb in nc.bb_map.items():
        insts = list(bb.bb.instructions)
        out, changed = [], False
        for i in insts:
            si = i.sync_info
            cap = caps.get(type(i).__name__, 1)
            if si is not None and len(si.on_wait) > cap:
                waits = list(si.on_wait)
                for k, w in enumerate(waits[cap:]):
                    nop = mybir.InstNoOp(name=f"{i.name}-ws{k}", engine=i.engine)
                    nop.sync_info = mybir.SyncInfo(on_wait=[w], on_update=[])
                    nc.register_instruction(nop)
                    out.append(nop)
                i.sync_info = mybir.SyncInfo(
                    on_wait=waits[:cap], on_update=list(si.on_update)
                )
                changed = True
            out.append(i)
        if changed:
            bb.bb.instructions = out


def build():
    nc = bass.Bass("TRN2", target_bir_lowering=False, debug=False, num_devices=1)
    x = nc.dram_tensor("x", [T, H], F32, kind="ExternalInput")
    w = nc.dram_tensor("w", [E, H], F32, kind="ExternalInput")
    ident = nc.dram_tensor("ident", [P, P], F32, kind="ExternalInput")
    ones = nc.dram_tensor("ones", [P, 1], F32, kind="ExternalInput")
    o_idx = nc.dram_tensor("o_idx", [T, K], I32, kind="ExternalOutput")
    o_wgt = nc.dram_tensor("o_wgt", [T, K], F32, kind="ExternalOutput")
    o_aux = nc.dram_tensor("o_aux", [1, NB * 2 * E], F32, kind="ExternalOutput")

    NCH = H // P  # 4 h-chunks
    GT = 4  # token-tiles per matmul group (N = GT*P = 512 moving cols)
    NG = NT // GT  # 16 groups
    x_dram = x.ap().rearrange("(p i) h -> p i h", p=P)  # [128, 64, 512]

    with tile.TileContext(nc) as tc:
        with (
            tc.tile_pool(name="consts", bufs=1) as consts,
            tc.tile_pool(name="xin", bufs=4) as xin,
            tc.tile_pool(name="xtp", bufs=3, space="PSUM") as xtp,
            tc.tile_pool(name="xtg", bufs=2) as xtg,
            tc.tile_pool(name="lgtp", bufs=2, space="PSUM") as lgtp,
            tc.tile_pool(name="lgts", bufs=2) as lgts,
            tc.tile_pool(name="lgtok", bufs=2, space="PSUM") as lgtokp,
            tc.tile_pool(name="auxp", bufs=1, space="PSUM") as auxp,
            tc.tile_pool(name="ph2", bufs=1) as ph2,
            tc.tile_pool(name="ph2t", bufs=1) as ph2t,
        ):
            # identity first (warm-up depends only on it), then the first x
            # chunk so its transfer overlaps everything else
            id_sb = consts.tile([P, P], F32)
            nc.sync.dma_start(id_sb[:], ident[:])
            x_ld0 = xin.tile([P, GT * H], F32, name="x_ld")
            nc.sync.dma_start(
                x_ld0[:].rearrange("p (i h) -> p i h", i=GT),
                x_dram[:, 0:GT, :],
            )
            ones_sb = consts.tile([P, 1], F32)
            nc.sync.dma_start(ones_sb[:], ones[:])
            # W^T chunk c ([128 h, 4 e]) lives at cols 4c..4c+3
            wt_sb = consts.tile([P, E * NCH], F32)
            wr = w.ap().rearrange("e h -> h e")
            for c in range(NCH):
                nc.sync.dma_start(
                    wt_sb[:, E * c : E * (c + 1)], wr[P * c : P * (c + 1), :]
                )

            # persistent SBUF state
            logits_all = ph2.tile([P, E * NT], F32)  # plane e at cols 64e+i
            aux_all = ph2.tile([P, NB * 2 * E], F32)  # per-batch partials
            out_idx = ph2.tile([P, NT * K], I32)
            out_wgt = ph2.tile([P, NT * K], F32)

            # HAM warm-up: junk matmuls depending only on id_sb; they run
            # while the first x DMA streams in.
            warm_ps = lgtokp.tile([P, E * LG_BATCH], F32, name="warm", tag="lgtok")
            for r in range(24):
                nc.tensor.matmul(
                    warm_ps[0:E, :],
                    id_sb[:, 0:E],
                    id_sb[:, 0 : E * LG_BATCH],
                    start=(r == 0),
                    stop=(r == 23),
                )

            def pl(e, b):
                return logits_all[:, NT * e + Bq * b : NT * e + Bq * (b + 1)]

            def phase2_batch(b):
                S = [P, Bq]
                A, B_, C_, D_ = (pl(e, b) for e in range(E))
                mx_ab = ph2t.tile(S, F32, name="mx_ab")
                nc.vector.tensor_tensor(mx_ab[:], A, B_, Alu.max)
                mx_cd = ph2t.tile(S, F32, name="mx_cd")
                nc.vector.tensor_tensor(mx_cd[:], C_, D_, Alu.max)
                mn_ab = ph2t.tile(S, F32, name="mn_ab")
                nc.vector.tensor_tensor(mn_ab[:], A, B_, Alu.min)
                mn_cd = ph2t.tile(S, F32, name="mn_cd")
                nc.vector.tensor_tensor(mn_cd[:], C_, D_, Alu.min)
                m1 = ph2t.tile(S, F32, name="m1")
                nc.vector.tensor_tensor(m1[:], mx_ab[:], mx_cd[:], Alu.max)
                t1 = ph2t.tile(S, F32, name="t1")
                nc.vector.tensor_tensor(t1[:], mx_ab[:], mx_cd[:], Alu.min)
                t2 = ph2t.tile(S, F32, name="t2")
                nc.vector.tensor_tensor(t2[:], mn_ab[:], mn_cd[:], Alu.max)
                m2 = ph2t.tile(S, F32, name="m2")
                nc.vector.tensor_tensor(m2[:], t1[:], t2[:], Alu.max)

                eq1 = [ph2t.tile(S, F32, name=f"eq1_{e}") for e in range(E)]
                eq2 = [ph2t.tile(S, F32, name=f"eq2_{e}") for e in range(E)]
                for e in range(E):
                    nc.vector.tensor_tensor(eq1[e][:], pl(e, b), m1[:], Alu.is_equal)
                    nc.vector.tensor_tensor(eq2[e][:], pl(e, b), m2[:], Alu.is_equal)

                # idx = eqB + 2*eqC + 3*eqD (exact small floats)
                idx0 = ph2t.tile(S, F32, name="idx0")
                idx1 = ph2t.tile(S, F32, name="idx1")
                tmp = ph2t.tile(S, F32, name="tmp")
                nc.vector.scalar_tensor_tensor(
                    tmp[:], eq1[2][:], 2.0, eq1[1][:], Alu.mult, Alu.add
                )
                nc.vector.scalar_tensor_tensor(
                    idx0[:], eq1[3][:], 3.0, tmp[:], Alu.mult, Alu.add
                )
                tmp2 = ph2t.tile(S, F32, name="tmp2")
                nc.vector.scalar_tensor_tensor(
                    tmp2[:], eq2[2][:], 2.0, eq2[1][:], Alu.mult, Alu.add
                )
                nc.vector.scalar_tensor_tensor(
                    idx1[:], eq2[3][:], 3.0, tmp2[:], Alu.mult, Alu.add
                )

                # weights: w0 = 1/(1+exp(m2-m1)), w1 = exp(m2-m1)*w0
                d21 = ph2t.tile(S, F32, name="d21")
                nc.vector.tensor_tensor(d21[:], m2[:], m1[:], Alu.subtract)
                e2 = ph2t.tile(S, F32, name="e2")
                nc.scalar.activation(e2[:], d21[:], Act.Exp)
                den = ph2t.tile(S, F32, name="den")
                nc.vector.tensor_scalar_add(den[:], e2[:], 1.0)
                w0 = ph2t.tile(S, F32, name="w0")
                nc.vector.reciprocal(w0[:], den[:])
                w1 = ph2t.tile(S, F32, name="w1")
                nc.vector.tensor_tensor(w1[:], e2[:], w0[:], Alu.mult)

                # softmax scores for aux
                expb = ph2t.tile([P, E * Bq], F32, name="expb")
                for e in range(E):
                    nc.vector.tensor_tensor(
                        expb[:, Bq * e : Bq * (e + 1)], pl(e, b), m1[:], Alu.subtract
                    )
                nc.scalar.activation(expb[:], expb[:], Act.Exp)

                def ex(e):
                    return expb[:, Bq * e : Bq * (e + 1)]

                z01 = ph2t.tile(S, F32, name="z01")
                nc.vector.tensor_tensor(z01[:], ex(0), ex(1), Alu.add)
                z23 = ph2t.tile(S, F32, name="z23")
                nc.vector.tensor_tensor(z23[:], ex(2), ex(3), Alu.add)
                zz = ph2t.tile(S, F32, name="zz")
                nc.vector.tensor_tensor(zz[:], z01[:], z23[:], Alu.add)
                rz = ph2t.tile(S, F32, name="rz")
                nc.vector.reciprocal(rz[:], zz[:])

                junk = ph2t.tile(S, F32, name="junk")
                a0 = NB_AUX * b
                for e in range(E):
                    nc.vector.scalar_tensor_tensor(
                        junk[:], ex(e), 1.0, rz[:], Alu.mult, Alu.mult,
                        accum_out=aux_all[:, a0 + e : a0 + e + 1],
                    )
                    nc.vector.scalar_tensor_tensor(
                        junk[:], eq1[e][:], 1.0, eq2[e][:], Alu.mult, Alu.add,
                        accum_out=aux_all[:, a0 + E + e : a0 + E + e + 1],
                    )

                # interleave outputs for this batch
                oi = out_idx[:].rearrange("p (t k) -> p t k", k=K)[
                    :, Bq * b : Bq * (b + 1), :
                ]
                ow = out_wgt[:].rearrange("p (t k) -> p t k", k=K)[
                    :, Bq * b : Bq * (b + 1), :
                ]
                nc.vector.tensor_copy(oi[:, :, 0], idx0[:])
                nc.vector.tensor_copy(oi[:, :, 1], idx1[:])
                nc.vector.tensor_copy(ow[:, :, 0], w0[:])
                nc.vector.tensor_copy(ow[:, :, 1], w1[:])

            lgtok_ps = None
            pending_bt = []  # deferred back-transposes: (group, lgT_sb tile)

            def emit_bt(g, lgt_s):
                nonlocal lgtok_ps
                for j in range(GT):
                    i = GT * g + j  # token-tile index
                    m = i % LG_BATCH
                    if m == 0:
                        lgtok_ps = lgtokp.tile(
                            [P, E * LG_BATCH], F32, name="lgtok_ps", tag="lgtok"
                        )
                    nc.tensor.matmul(
                        lgtok_ps[:, E * m : E * (m + 1)],
                        lgt_s[0:E, P * j : P * (j + 1)],
                        id_sb[0:E, 0:E],
                        is_transpose=True,
                        start=True,
                        stop=True,
                    )
                    if m == LG_BATCH - 1:
                        b = i // LG_BATCH
                        src = lgtok_ps[:].rearrange("p (t e) -> p e t", e=E)
                        dst = logits_all[:].rearrange("p (e t) -> p e t", e=E)[
                            :, :, LG_BATCH * b : LG_BATCH * (b + 1)
                        ]
                        if b % 2 == 0:
                            nc.vector.tensor_copy(dst, src)
                        else:
                            nc.scalar.copy(dst, src)
                        phase2_batch(b)

            for g in range(NG):
                if g == 0:
                    x_ld = x_ld0
                else:
                    x_ld = xin.tile([P, GT * H], F32, name="x_ld")
                    nc.sync.dma_start(
                        x_ld[:].rearrange("p (i h) -> p i h", i=GT),
                        x_dram[:, GT * g : GT * (g + 1), :],
                    )
                # chunk-major xT for this group: block (c, j) at cols 512c+128j
                xt_g = xtg.tile([P, NCH * GT * P], F32)
                xt_g3 = xt_g[:].rearrange("p (c t) -> p c t", c=NCH)
                for j in range(GT):
                    xt_p = xtp.tile([P, H], F32)
                    for c in range(NCH):
                        nc.tensor.matmul(
                            xt_p[:, P * c : P * (c + 1)],
                            x_ld[:, H * j + P * c : H * j + P * (c + 1)],
                            id_sb[:],
                            is_transpose=True,
                            start=(c == 0),
                            stop=(c == NCH - 1),
                        )
                    xt_p3 = xt_p[:].rearrange("p (c t) -> p c t", c=NCH)
                    nc.vector.tensor_copy(
                        xt_g3[:, 0:2, P * j : P * (j + 1)], xt_p3[:, 0:2, :]
                    )
                    nc.scalar.copy(
                        xt_g3[:, 2:4, P * j : P * (j + 1)], xt_p3[:, 2:4, :]
                    )

                # back-transposes of the PREVIOUS group sit here in PE order so
                # they never stall on the lgT copy
                if pending_bt:
                    emit_bt(*pending_bt.pop())

                lgt_p = lgtp.tile([E, GT * P], F32)
                for c in range(NCH):
                    nc.tensor.matmul(
                        lgt_p[:],
                        wt_sb[:, E * c : E * (c + 1)],
                        xt_g[:, GT * P * c : GT * P * (c + 1)],
                        start=(c == 0),
                        stop=(c == NCH - 1),
                    )
                lgt_s = lgts.tile([E, GT * P], F32)
                if g % 2 == 0:
                    nc.vector.tensor_copy(lgt_s[:], lgt_p[:])
                else:
                    nc.scalar.copy(lgt_s[:], lgt_p[:])
                pending_bt.append((g, lgt_s))

            while pending_bt:
                emit_bt(*pending_bt.pop())

            # final: cross-partition reduce of aux partials, then stores
            aux_ps = auxp.tile([1, NB * 2 * E], F32)
            nc.tensor.matmul(aux_ps[:], ones_sb[:], aux_all[:], start=True, stop=True)
            aux_out = ph2.tile([1, NB * 2 * E], F32)
            nc.vector.tensor_copy(aux_out[:], aux_ps[:])
            nc.sync.dma_start(o_aux[:], aux_out[:])
            nc.sync.dma_start(
                o_idx.ap().rearrange("(p t) k -> p t k", p=P),
                out_idx[:].rearrange("p (t k) -> p t k", k=K),
            )
            nc.sync.dma_start(
                o_wgt.ap().rearrange("(p t) k -> p t k", p=P),
                out_wgt[:].rearrange("p (t k) -> p t k", k=K),
            )
    _split_excess_waits(nc)
    return nc


_NC_CACHE = None


def _get_nc():
    global _NC_CACHE
    if _NC_CACHE is None:
        _NC_CACHE = build()
    return _NC_CACHE


def _in_maps(hidden_states, weight):
    ident = np.eye(P, dtype=np.float32)
    ones = np.ones((P, 1), dtype=np.float32)
    maps = []
    for c in range(N_CORES):
        # token t of core c = batch row c, seq p*64+i
        maps.append(
            {
                "x": np.ascontiguousarray(hidden_states[c]),
                "w": np.ascontiguousarray(weight),
                "ident": ident,
                "ones": ones,
            }
        )
    return maps


def _combine(results):
    idx = np.concatenate([r["o_idx"] for r in results], axis=0)
    wgt = np.concatenate([r["o_wgt"] for r in results], axis=0)
    aux_parts = np.stack(
        [r["o_aux"][0].reshape(NB, 2 * E).sum(axis=0) for r in results], axis=0
    )  # [8, 8]
    scores_sum = aux_parts[:, :E].astype(np.float64)  # [8, 4]
    counts = aux_parts[:, E:].astype(np.float64)  # [8, 4]
    ce = counts / (SEQ * K / E)
    pi = scores_sum / SEQ
    aux = np.float32(ALPHA * np.mean(np.sum(ce * pi, axis=1)))
    return idx.astype(np.int32), wgt.astype(np.float32), np.asarray(aux, np.float32)


def run(hidden_states, weight, trace=False, **spmd_kwargs):
    nc = _get_nc()
    res = run_bass_kernel_spmd(
        nc,
        _in_maps(hidden_states, weight),
        core_ids=list(range(N_CORES)),
        trace=trace,
        **spmd_kwargs,
    )
    out = _combine(res.results)
    return out, res


def kernel(hidden_states, weight):
    out, _ = run(hidden_states, weight)
    return out


# revision 10
# speedup vs baseline: 1.3924x; 1.3924x over previous
"""MoE gate kernel for TRN2 (8 NeuronCores, Bass/Tile).

Computes, for hidden_states [8, 8192, 512] f32 and gate weight [4, 512] f32:
  logits = x @ W^T, scores = softmax(logits), top-2 (values normalized) and
  the seq-aux load-balancing loss — matching the reference MoEGate module.

Sharding: data-parallel over the batch dim — core c handles batch row c
(8192 tokens). The tiny weight is replicated. The scalar aux_loss partials
([scores_colsum, expert_counts] per core) are combined on the host.

Per-core dataflow, token t = p*64 + i (p = SBUF partition, i = tile 0..63):
  phase 1 (per 128-token tile): DMA x in 1 MiB chunks -> PE transpose chunks
  [128t,128h] -> [128h,128t] (fp32 transpose is bit-exact) -> 4 accumulating
  fp32 matmuls against W^T chunks -> logits [128,4] in PSUM -> batched copy
  into an SBUF plane buffer logits_all [128, 4*64] (plane e at cols 64e+i).
  phase 2 (all tokens at once, [128,64] plane ops): top-2 via min/max
  identities, argmax via is_equal masks, weights via exp + reciprocal,
  softmax column-sums and expert counts reduced per partition, then one
  ones-vector matmul to reduce across partitions.
"""

import numpy as np

import concourse.bass as bass
import concourse.tile as tile
from concourse import mybir
from concourse.bass_utils import run_bass_kernel_spmd

N_CORES = 8
BSZ, SEQ, H = 8, 8192, 512
E = 4  # experts
K = 2  # top-k
ALPHA = 0.1
T = SEQ  # tokens per core
P = 128  # partitions
NT = T // P  # 64 token-tiles per core
DMA_GROUP = 4  # token-tiles per input DMA (1 MiB)
LG_BATCH = 16  # token-tiles per PSUM logits batch
NB = NT // LG_BATCH  # 4 phase-2 batches
Bq = LG_BATCH  # columns per batch in plane space
NB_AUX = 2 * 4  # aux partial cols per batch

F32 = mybir.dt.float32
BF16 = mybir.dt.bfloat16
I32 = mybir.dt.int32
Alu = mybir.AluOpType
Act = mybir.ActivationFunctionType


def _split_excess_waits(nc):
    """walrus in this container allows 1 sync wait per instruction (2 for
    EventSemaphore); Tile's final drain can carry more. Move extras to NOPs."""
    caps = {"InstEventSemaphore": 2}
    for bbname, bb in nc.bb_map.items():
        insts = list(bb.bb.instructions)
        out, changed = [], False
        for i in insts:
            si = i.sync_info
            cap = caps.get(type(i).__name__, 1)
            if si is not None and len(si.on_wait) > cap:
                waits = list(si.on_wait)
                for k, w in enumerate(waits[cap:]):
                    nop = mybir.InstNoOp(name=f"{i.name}-ws{k}", engine=i.engine)
                    nop.sync_info = mybir.SyncInfo(on_wait=[w], on_update=[])
                    nc.register_instruction(nop)
                    out.append(nop)
                i.sync_info = mybir.SyncInfo(
                    on_wait=waits[:cap], on_update=list(si.on_update)
                )
                changed = True
            out.append(i)
        if changed:
            bb.bb.instructions = out


def build():
    nc = bass.Bass("TRN2", target_bir_lowering=False, debug=False, num_devices=1)
    x = nc.dram_tensor("x", [T, H], F32, kind="ExternalInput")
    w = nc.dram_tensor("w", [E, H], F32, kind="ExternalInput")
    ident = nc.dram_tensor("ident", [P, P], F32, kind="ExternalInput")
    ones = nc.dram_tensor("ones", [P, 1], F32, kind="ExternalInput")
    o_idx = nc.dram_tensor("o_idx", [T, K], I32, kind="ExternalOutput")
    o_wgt = nc.dram_tensor("o_wgt", [T, K], F32, kind="ExternalOutput")
    o_aux = nc.dram_tensor("o_aux", [1, NB * 2 * E], F32, kind="ExternalOutput")
    o_mgn = nc.dram_tensor("o_mgn", [T, K], F32, kind="ExternalOutput")

    NCH = H // P  # 4 h-chunks
    GT = 4  # token-tiles per matmul group (N = GT*P = 512 moving cols)
    NG = NT // GT  # 16 groups
    x_dram = x.ap().rearrange("(p i) h -> p i h", p=P)  # [128, 64, 512]

    with tile.TileContext(nc) as tc:
        with (
            tc.tile_pool(name="consts", bufs=1) as consts,
            tc.tile_pool(name="xin", bufs=4) as xin,
            tc.tile_pool(name="xtp", bufs=3, space="PSUM") as xtp,
            tc.tile_pool(name="xtg", bufs=2) as xtg,
            tc.tile_pool(name="lgtp", bufs=2, space="PSUM") as lgtp,
            tc.tile_pool(name="lgts", bufs=2) as lgts,
            tc.tile_pool(name="lgtok", bufs=2, space="PSUM") as lgtokp,
            tc.tile_pool(name="auxp", bufs=1, space="PSUM") as auxp,
            tc.tile_pool(name="ph2", bufs=1) as ph2,
            tc.tile_pool(name="ph2t", bufs=1) as ph2t,
        ):
            # identity first (warm-up depends only on it), then the first x
            # chunk so its transfer overlaps everything else
            id_sb = consts.tile([P, P], F32)
            nc.sync.dma_start(id_sb[:], ident[:])
            x_ld0 = xin.tile([P, GT * H], F32, name="x_ld")
            nc.sync.dma_start(
                x_ld0[:].rearrange("p (i h) -> p i h", i=GT),
                x_dram[:, 0:GT, :],
            )
            ones_sb = consts.tile([P, 1], F32)
            nc.sync.dma_start(ones_sb[:], ones[:])
            # W^T chunk c ([128 h, 4 e]) lives at cols 4c..4c+3
            wt_sb = consts.tile([P, E * NCH], F32)
            wr = w.ap().rearrange("e h -> h e")
            for c in range(NCH):
                nc.sync.dma_start(
                    wt_sb[:, E * c : E * (c + 1)], wr[P * c : P * (c + 1), :]
                )
            # bf16 hi/lo split of W^T: chunk c occupies cols 8c..8c+7 =
            # [Wh_c (4) | Wl_c (4)]
            whl_sb = consts.tile([P, 2 * E * NCH], BF16)
            whl3 = whl_sb[:].rearrange("p (c two e) -> p c two e", c=NCH, two=2)
            wt3v = wt_sb[:].rearrange("p (c e) -> p c e", c=NCH)
            nc.scalar.copy(whl3[:, :, 0, :], wt3v)
            nc.vector.tensor_tensor(
                whl3[:, :, 1, :], wt3v, whl3[:, :, 0, :], Alu.subtract
            )

            # persistent SBUF state
            logits_all = ph2.tile([P, E * NT], F32)  # plane e at cols 64e+i
            aux_all = ph2.tile([P, NB * 2 * E], F32)  # per-batch partials
            out_idx = ph2.tile([P, NT * K], I32)
            out_wgt = ph2.tile([P, NT * K], F32)
            out_mgn = ph2.tile([P, NT * K], F32)

            # HAM warm-up: junk matmuls depending only on id_sb; they run
            # while the first x DMA streams in.
            warm_ps = lgtokp.tile([P, E * LG_BATCH], F32, name="warm", tag="lgtok")
            for r in range(24):
                nc.tensor.matmul(
                    warm_ps[0:E, :],
                    id_sb[:, 0:E],
                    id_sb[:, 0 : E * LG_BATCH],
                    start=(r == 0),
                    stop=(r == 23),
                )

            def pl(e, b):
                return logits_all[:, NT * e + Bq * b : NT * e + Bq * (b + 1)]

            def phase2_batch(b):
                S = [P, Bq]
                A, B_, C_, D_ = (pl(e, b) for e in range(E))
                mx_ab = ph2t.tile(S, F32, name="mx_ab")
                nc.vector.tensor_tensor(mx_ab[:], A, B_, Alu.max)
                mx_cd = ph2t.tile(S, F32, name="mx_cd")
                nc.vector.tensor_tensor(mx_cd[:], C_, D_, Alu.max)
                mn_ab = ph2t.tile(S, F32, name="mn_ab")
                nc.vector.tensor_tensor(mn_ab[:], A, B_, Alu.min)
                mn_cd = ph2t.tile(S, F32, name="mn_cd")
                nc.vector.tensor_tensor(mn_cd[:], C_, D_, Alu.min)
                m1 = ph2t.tile(S, F32, name="m1")
                nc.vector.tensor_tensor(m1[:], mx_ab[:], mx_cd[:], Alu.max)
                t1 = ph2t.tile(S, F32, name="t1")
                nc.vector.tensor_tensor(t1[:], mx_ab[:], mx_cd[:], Alu.min)
                t2 = ph2t.tile(S, F32, name="t2")
                nc.vector.tensor_tensor(t2[:], mn_ab[:], mn_cd[:], Alu.max)
                m2 = ph2t.tile(S, F32, name="m2")
                nc.vector.tensor_tensor(m2[:], t1[:], t2[:], Alu.max)
                m3 = ph2t.tile(S, F32, name="m3")
                nc.vector.tensor_tensor(m3[:], t1[:], t2[:], Alu.min)

                eq1 = [ph2t.tile(S, F32, name=f"eq1_{e}") for e in range(E)]
                eq2 = [ph2t.tile(S, F32, name=f"eq2_{e}") for e in range(E)]
                for e in range(E):
                    nc.vector.tensor_tensor(eq1[e][:], pl(e, b), m1[:], Alu.is_equal)
                    nc.vector.tensor_tensor(eq2[e][:], pl(e, b), m2[:], Alu.is_equal)

                # idx = eqB + 2*eqC + 3*eqD (exact small floats)
                idx0 = ph2t.tile(S, F32, name="idx0")
                idx1 = ph2t.tile(S, F32, name="idx1")
                tmp = ph2t.tile(S, F32, name="tmp")
                nc.vector.scalar_tensor_tensor(
                    tmp[:], eq1[2][:], 2.0, eq1[1][:], Alu.mult, Alu.add
                )
                nc.vector.scalar_tensor_tensor(
                    idx0[:], eq1[3][:], 3.0, tmp[:], Alu.mult, Alu.add
                )
                tmp2 = ph2t.tile(S, F32, name="tmp2")
                nc.vector.scalar_tensor_tensor(
                    tmp2[:], eq2[2][:], 2.0, eq2[1][:], Alu.mult, Alu.add
                )
                nc.vector.scalar_tensor_tensor(
                    idx1[:], eq2[3][:], 3.0, tmp2[:], Alu.mult, Alu.add
                )

                # weights: w0 = 1/(1+exp(m2-m1)), w1 = exp(m2-m1)*w0
                d21 = ph2t.tile(S, F32, name="d21")
                nc.vector.tensor_tensor(d21[:], m2[:], m1[:], Alu.subtract)
                e2 = ph2t.tile(S, F32, name="e2")
                nc.scalar.activation(e2[:], d21[:], Act.Exp)
                den = ph2t.tile(S, F32, name="den")
                nc.vector.tensor_scalar_add(den[:], e2[:], 1.0)
                w0 = ph2t.tile(S, F32, name="w0")
                nc.vector.reciprocal(w0[:], den[:])
                w1 = ph2t.tile(S, F32, name="w1")
                nc.vector.tensor_tensor(w1[:], e2[:], w0[:], Alu.mult)

                # softmax scores for aux
                expb = ph2t.tile([P, E * Bq], F32, name="expb")
                for e in range(E):
                    nc.vector.tensor_tensor(
                        expb[:, Bq * e : Bq * (e + 1)], pl(e, b), m1[:], Alu.subtract
                    )
                nc.scalar.activation(expb[:], expb[:], Act.Exp)

                def ex(e):
                    return expb[:, Bq * e : Bq * (e + 1)]

                z01 = ph2t.tile(S, F32, name="z01")
                nc.vector.tensor_tensor(z01[:], ex(0), ex(1), Alu.add)
                z23 = ph2t.tile(S, F32, name="z23")
                nc.vector.tensor_tensor(z23[:], ex(2), ex(3), Alu.add)
                zz = ph2t.tile(S, F32, name="zz")
                nc.vector.tensor_tensor(zz[:], z01[:], z23[:], Alu.add)
                rz = ph2t.tile(S, F32, name="rz")
                nc.vector.reciprocal(rz[:], zz[:])

                junk = ph2t.tile(S, F32, name="junk")
                a0 = NB_AUX * b
                for e in range(E):
                    nc.vector.scalar_tensor_tensor(
                        junk[:], ex(e), 1.0, rz[:], Alu.mult, Alu.mult,
                        accum_out=aux_all[:, a0 + e : a0 + e + 1],
                    )
                    nc.vector.scalar_tensor_tensor(
                        junk[:], eq1[e][:], 1.0, eq2[e][:], Alu.mult, Alu.add,
                        accum_out=aux_all[:, a0 + E + e : a0 + E + e + 1],
                    )

                # margins for host-side exact re-check of near-ties
                g12 = ph2t.tile(S, F32, name="g12")
                nc.vector.tensor_tensor(g12[:], m1[:], m2[:], Alu.subtract)
                g23 = ph2t.tile(S, F32, name="g23")
                nc.vector.tensor_tensor(g23[:], m2[:], m3[:], Alu.subtract)
                om = out_mgn[:].rearrange("p (t k) -> p t k", k=K)[
                    :, Bq * b : Bq * (b + 1), :
                ]
                nc.scalar.copy(om[:, :, 0], g12[:])
                nc.scalar.copy(om[:, :, 1], g23[:])

                # interleave outputs for this batch
                oi = out_idx[:].rearrange("p (t k) -> p t k", k=K)[
                    :, Bq * b : Bq * (b + 1), :
                ]
                ow = out_wgt[:].rearrange("p (t k) -> p t k", k=K)[
                    :, Bq * b : Bq * (b + 1), :
                ]
                nc.vector.tensor_copy(oi[:, :, 0], idx0[:])
                nc.vector.tensor_copy(oi[:, :, 1], idx1[:])
                nc.vector.tensor_copy(ow[:, :, 0], w0[:])
                nc.vector.tensor_copy(ow[:, :, 1], w1[:])

            lgtok_ps = None
            pending_bt = []  # deferred back-transposes: (group, lgT_sb tile)

            def emit_bt(g, lgt_s):
                nonlocal lgtok_ps
                for j in range(GT):
                    i = GT * g + j  # token-tile index
                    m = i % LG_BATCH
                    if m == 0:
                        lgtok_ps = lgtokp.tile(
                            [P, 2 * E * LG_BATCH], F32, name="lgtok_ps", tag="lgtok"
                        )
                    nc.tensor.matmul(
                        lgtok_ps[:, 2 * E * m : 2 * E * (m + 1)],
                        lgt_s[:, P * j : P * (j + 1)],
                        id_sb[0 : 2 * E, 0 : 2 * E],
                        is_transpose=True,
                        start=True,
                        stop=True,
                    )
                    if m == LG_BATCH - 1:
                        b = i // LG_BATCH
                        # PSUM -> SBUF, then logits = hi rows + lo rows
                        lgtok_sb = ph2t.tile(
                            [P, 2 * E * LG_BATCH], F32, name="lgtok_sb"
                        )
                        nc.scalar.copy(lgtok_sb[:], lgtok_ps[:])
                        lv = lgtok_sb[:].rearrange(
                            "p (t two e) -> p two e t", two=2, e=E
                        )
                        dst = logits_all[:].rearrange("p (e t) -> p e t", e=E)[
                            :, :, LG_BATCH * b : LG_BATCH * (b + 1)
                        ]
                        nc.vector.tensor_tensor(
                            dst, lv[:, 0, :, :], lv[:, 1, :, :], Alu.add
                        )
                        phase2_batch(b)

            for g in range(NG):
                if g == 0:
                    x_ld = x_ld0
                else:
                    x_ld = xin.tile([P, GT * H], F32, name="x_ld")
                    nc.sync.dma_start(
                        x_ld[:].rearrange("p (i h) -> p i h", i=GT),
                        x_dram[:, GT * g : GT * (g + 1), :],
                    )
                # chunk-major bf16 hi/lo xT for this group:
                # block (c, j) at cols 512c+128j of each of xh_g / xl_g
                xh_g = xtg.tile([P, NCH * GT * P], BF16, name="xh_g")
                xl_g = xtg.tile([P, NCH * GT * P], BF16, name="xl_g")
                xh_g3 = xh_g[:].rearrange("p (c t) -> p c t", c=NCH)
                xl_g3 = xl_g[:].rearrange("p (c t) -> p c t", c=NCH)
                for j in range(GT):
                    xt_p = xtp.tile([P, H], F32)
                    for c in range(NCH):
                        nc.tensor.matmul(
                            xt_p[:, P * c : P * (c + 1)],
                            x_ld[:, H * j + P * c : H * j + P * (c + 1)],
                            id_sb[:],
                            is_transpose=True,
                            start=(c == 0),
                            stop=(c == NCH - 1),
                        )
                    xt_p3 = xt_p[:].rearrange("p (c t) -> p c t", c=NCH)
                    nc.scalar.copy(xh_g3[:, :, P * j : P * (j + 1)], xt_p3)
                    nc.vector.tensor_tensor(
                        xl_g3[:, :, P * j : P * (j + 1)],
                        xt_p3,
                        xh_g3[:, :, P * j : P * (j + 1)],
                        Alu.subtract,
                    )

                # back-transposes of the PREVIOUS group sit here in PE order so
                # they never stall on the lgT copy
                if pending_bt:
                    emit_bt(*pending_bt.pop())

                lgt_p = lgtp.tile([2 * E, GT * P], F32)
                for c in range(NCH):
                    nc.tensor.matmul(
                        lgt_p[:],
                        whl_sb[:, 2 * E * c : 2 * E * (c + 1)],
                        xh_g[:, GT * P * c : GT * P * (c + 1)],
                        start=(c == 0),
                        stop=False,
                    )
                for c in range(NCH):
                    nc.tensor.matmul(
                        lgt_p[:],
                        whl_sb[:, 2 * E * c : 2 * E * (c + 1)],
                        xl_g[:, GT * P * c : GT * P * (c + 1)],
                        start=False,
                        stop=(c == NCH - 1),
                    )
                lgt_s = lgts.tile([2 * E, GT * P], F32)
                if g % 2 == 0:
                    nc.vector.tensor_copy(lgt_s[:], lgt_p[:])
                else:
                    nc.scalar.copy(lgt_s[:], lgt_p[:])
                pending_bt.append((g, lgt_s))

            while pending_bt:
                emit_bt(*pending_bt.pop())

            # final: cross-partition reduce of aux partials, then stores
            aux_ps = auxp.tile([1, NB * 2 * E], F32)
            nc.tensor.matmul(aux_ps[:], ones_sb[:], aux_all[:], start=True, stop=True)
            aux_out = ph2.tile([1, NB * 2 * E], F32)
            nc.vector.tensor_copy(aux_out[:], aux_ps[:])
            nc.sync.dma_start(o_aux[:], aux_out[:])
            nc.sync.dma_start(
                o_idx.ap().rearrange("(p t) k -> p t k", p=P),
                out_idx[:].rearrange("p (t k) -> p t k", k=K),
            )
            nc.sync.dma_start(
                o_wgt.ap().rearrange("(p t) k -> p t k", p=P),
                out_wgt[:].rearrange("p (t k) -> p t k", k=K),
            )
            nc.sync.dma_start(
                o_mgn.ap().rearrange("(p t) k -> p t k", p=P),
                out_mgn[:].rearrange("p (t k) -> p t k", k=K),
            )
    _split_excess_waits(nc)
    return nc


_NC_CACHE = None


def _get_nc():
    global _NC_CACHE
    if _NC_CACHE is None:
        _NC_CACHE = build()
    return _NC_CACHE


def _in_maps(hidden_states, weight):
    ident = np.eye(P, dtype=np.float32)
    ones = np.ones((P, 1), dtype=np.float32)
    maps = []
    for c in range(N_CORES):
        # token t of core c = batch row c, seq p*64+i
        maps.append(
            {
                "x": np.ascontiguousarray(hidden_states[c]),
                "w": np.ascontiguousarray(weight),
                "ident": ident,
                "ones": ones,
            }
        )
    return maps


TAU = 2e-3  # >> worst-case bf16-split logit error (~8e-4); near-ties get
# recomputed exactly on the host


def _patch(results, hidden_states, weight):
    """Recompute near-tie tokens exactly; patch idx/wgt and count deltas."""
    w64 = weight.astype(np.float64)
    count_deltas = []
    for c, r in enumerate(results):
        mgn = r["o_mgn"]
        sus = np.where(np.minimum(mgn[:, 0], mgn[:, 1]) < TAU)[0]
        dcount = np.zeros(E, np.float64)
        if sus.size:
            x64 = hidden_states[c].reshape(-1, H)[sus].astype(np.float64)
            lg = x64 @ w64.T
            order = np.argsort(-lg, axis=1, kind="stable")
            i0, i1 = order[:, 0], order[:, 1]
            sc = np.exp(lg - lg.max(axis=1, keepdims=True))
            sc /= sc.sum(axis=1, keepdims=True)
            s0 = sc[np.arange(sus.size), i0]
            s1 = sc[np.arange(sus.size), i1]
            den = s0 + s1
            old = r["o_idx"][sus]
            for e in range(E):
                dcount[e] += (i0 == e).sum() + (i1 == e).sum()
                dcount[e] -= (old[:, 0] == e).sum() + (old[:, 1] == e).sum()
            r["o_idx"] = r["o_idx"].copy()
            r["o_wgt"] = r["o_wgt"].copy()
            r["o_idx"][sus, 0] = i0
            r["o_idx"][sus, 1] = i1
            r["o_wgt"][sus, 0] = (s0 / den).astype(np.float32)
            r["o_wgt"][sus, 1] = (s1 / den).astype(np.float32)
        count_deltas.append(dcount)
    return count_deltas


def _combine(results, count_deltas=None):
    idx = np.concatenate([r["o_idx"] for r in results], axis=0)
    wgt = np.concatenate([r["o_wgt"] for r in results], axis=0)
    aux_parts = np.stack(
        [r["o_aux"][0].reshape(NB, 2 * E).sum(axis=0) for r in results], axis=0
    )  # [8, 8]
    scores_sum = aux_parts[:, :E].astype(np.float64)  # [8, 4]
    counts = aux_parts[:, E:].astype(np.float64)  # [8, 4]
    if count_deltas is not None:
        counts = counts + np.stack(count_deltas, axis=0)
    ce = counts / (SEQ * K / E)
    pi = scores_sum / SEQ
    aux = np.float32(ALPHA * np.mean(np.sum(ce * pi, axis=1)))
    return idx.astype(np.int32), wgt.astype(np.float32), np.asarray(aux, np.float32)


def run(hidden_states, weight, trace=False, **spmd_kwargs):
    nc = _get_nc()
    res = run_bass_kernel_spmd(
        nc,
        _in_maps(hidden_states, weight),
        core_ids=list(range(N_CORES)),
        trace=trace,
        **spmd_kwargs,
    )
    deltas = _patch(res.results, hidden_states, weight)
    out = _combine(res.results, deltas)
    return out, res


def kernel(hidden_states, weight):
    out, _ = run(hidden_states, weight)
    return out
